# revision 5
# baseline (speedup 1.0000x reference)
"""DeepSeek-V3 MLA attention (B=1, S=2048) on 8 TRN2 NeuronCores.

Sharding: tensor-parallel over heads (4 heads/core) for the up-projections,
attention and o_proj; the low-rank down-projections are token-sharded
(256 tokens/core) and exchanged with a single AllGather of the latents.
Each core returns a partial o_proj output (its 4 heads); the host sums them.

All on-chip matmuls run as float32r (full PE rate). Activations are kept
feature-major [feat, token]; softmax runs on transposed scores
S^T[tok_k, tok_q] so the reduction over keys is a ones-vector matmul.
"""

from contextlib import ExitStack

import numpy as np

import concourse.bass as bass
import concourse.tile as tile
from concourse import bacc, mybir
from concourse.bass_utils import run_bass_kernel_spmd

F32 = mybir.dt.float32
F32R = mybir.dt.float32r
AF = mybir.ActivationFunctionType

HIDDEN = 4096
N_HEADS = 32
Q_LORA = 1536
KV_LORA = 512
ROPE_D = 64
NOPE_D = 128
V_D = 128
QH = NOPE_D + ROPE_D  # 192
EPS = 1e-6
SCALING = QH ** -0.5
S = 2048
NCORE = 8
SC = S // NCORE  # 256 tokens per core chunk
HPC = N_HEADS // NCORE  # 4 heads per core

QKT = Q_LORA // 128  # 12
KVKT = KV_LORA // 128  # 4
HKT = HIDDEN // 128  # 32

# AllGather contribution rows: q_lat(1536) + kv_lat(512) + k_rot(64) + 2 rstd rows
KV0 = Q_LORA
KR0 = Q_LORA + KV_LORA
RSQ = KR0 + ROPE_D
RSKV = RSQ + 1
AGR = RSKV + 1  # 2114


def _cs(c):
    return slice(c * SC, (c + 1) * SC)


def _qs(qb):
    return slice(qb * 512, (qb + 1) * 512)


def _kts(kt):
    return slice(kt * 128, (kt + 1) * 128)


def build(debug=False):
    nc = bacc.Bacc(None, target_bir_lowering=False, num_devices=NCORE)

    def din(name, shape):
        return nc.dram_tensor(name, shape, F32, kind="ExternalInput")

    hidden_T = din("hidden_T", [HIDDEN, SC])
    w_q_a_T = din("w_q_a_T", [HIDDEN, Q_LORA])
    w_kv_a_T = din("w_kv_a_T", [HIDDEN, KV_LORA + ROPE_D])
    cos_c = din("cos_c", [ROPE_D, SC])
    sin_c = din("sin_c", [ROPE_D, SC])
    cos_T = din("cos_T", [ROPE_D, S])
    sin_T = din("sin_T", [ROPE_D, S])
    w_qb_nope_T = din("w_qb_nope_T", [Q_LORA, HPC * NOPE_D])
    w_qb_rot_T = din("w_qb_rot_T", [Q_LORA, HPC * ROPE_D])
    w_kb_T = din("w_kb_T", [KV_LORA, HPC * NOPE_D])
    w_vb_T = din("w_vb_T", [KV_LORA, HPC * V_D])
    w_o_T = din("w_o_T", [HPC * V_D, HIDDEN])
    rot64 = din("rot64", [ROPE_D, ROPE_D])
    ones_in = din("ones_in", [128, 1])
    eps_in = din("eps_in", [1, 1])
    mask_strip = din("mask_strip", [128, 896])

    o_partial = nc.dram_tensor("o_partial", [HIDDEN, S], F32, kind="ExternalOutput")
    dbg = {}
    if debug:
        dbg["ag"] = nc.dram_tensor("dbg_ag", [NCORE * AGR, SC], F32, kind="ExternalOutput")
        dbg["qnope"] = nc.dram_tensor("dbg_qnope", [NOPE_D, S], F32, kind="ExternalOutput")
        dbg["qrot"] = nc.dram_tensor("dbg_qrot", [ROPE_D, S], F32, kind="ExternalOutput")
        dbg["knope"] = nc.dram_tensor("dbg_knope", [NOPE_D, S], F32, kind="ExternalOutput")
        dbg["v"] = nc.dram_tensor("dbg_v", [128, 16 * 256], F32, kind="ExternalOutput")
        dbg["o"] = nc.dram_tensor("dbg_o", [V_D, S], F32, kind="ExternalOutput")

    ag_in = nc.dram_tensor("ag_in", [AGR, SC], F32, kind="Internal")
    ag_out = nc.dram_tensor(
        "ag_out", [NCORE * AGR, SC], F32, kind="Internal", addr_space="Shared"
    )

    with tile.TileContext(nc) as tc:
        # ---------------- phase 1: latents for own token chunk ----------------
        with (
            tc.tile_pool(name="p1", bufs=1) as p1,
            tc.tile_pool(name="p1s", bufs=8) as p1s,
            tc.tile_pool(name="ps1", bufs=2, space="PSUM") as ps1,
            tc.tile_pool(name="ps1a", bufs=1, space="PSUM") as ps1a,
        ):
            ht = p1.tile([128, HKT, SC], F32R)
            nc.sync.dma_start(
                ht[:], hidden_T[:].rearrange("(t p) n -> p t n", p=128).bitcast(F32R)
            )
            ones1 = p1.tile([128, 1], F32R)
            nc.sync.dma_start(ones1[:], ones_in[:].bitcast(F32R))
            epst = p1.tile([1, 1], F32)
            nc.sync.dma_start(epst[:], eps_in[:])

            ssq = ps1a.tile([1, SC], F32)
            sskv = ps1a.tile([1, SC], F32)

            # q_lat m-tiles
            for m in range(QKT):
                ps = ps1.tile([128, SC], F32, tag="pslat")
                for k in range(HKT):
                    wt = p1s.tile([128, 128], F32R, tag="wqa")
                    nc.sync.dma_start(
                        wt[:],
                        w_q_a_T[_kts(k), m * 128 : (m + 1) * 128].bitcast(F32R),
                    )
                    nc.tensor.matmul(
                        ps[:], wt[:], ht[:, k, :], start=(k == 0), stop=(k == HKT - 1)
                    )
                raw = p1s.tile([128, SC], F32, tag="lraw")
                nc.vector.tensor_copy(raw[:], ps[:])
                nc.sync.dma_start(ag_in[m * 128 : (m + 1) * 128, :], raw[:])
                sq = p1s.tile([128, SC], F32R, tag="lsq")
                nc.scalar.activation(sq[:], ps[:], AF.Square)
                nc.tensor.matmul(
                    ssq[:], ones1[:], sq[:], start=(m == 0), stop=(m == QKT - 1)
                )

            # kv_lat m-tiles
            for m in range(KVKT):
                ps = ps1.tile([128, SC], F32, tag="pslat")
                for k in range(HKT):
                    wt = p1s.tile([128, 128], F32R, tag="wqa")
                    nc.sync.dma_start(
                        wt[:],
                        w_kv_a_T[_kts(k), m * 128 : (m + 1) * 128].bitcast(F32R),
                    )
                    nc.tensor.matmul(
                        ps[:], wt[:], ht[:, k, :], start=(k == 0), stop=(k == HKT - 1)
                    )
                raw = p1s.tile([128, SC], F32, tag="lraw")
                nc.vector.tensor_copy(raw[:], ps[:])
                nc.sync.dma_start(ag_in[KV0 + m * 128 : KV0 + (m + 1) * 128, :], raw[:])
                sq = p1s.tile([128, SC], F32R, tag="lsq")
                nc.scalar.activation(sq[:], ps[:], AF.Square)
                nc.tensor.matmul(
                    sskv[:], ones1[:], sq[:], start=(m == 0), stop=(m == KVKT - 1)
                )

            # k_rot (rows 512:576 of ckv), roped with this chunk's cos/sin
            psr = ps1.tile([ROPE_D, SC], F32, tag="psrot")
            for k in range(HKT):
                wt = p1s.tile([128, ROPE_D], F32R, tag="wkvr")
                nc.sync.dma_start(
                    wt[:], w_kv_a_T[_kts(k), KV_LORA : KV_LORA + ROPE_D].bitcast(F32R)
                )
                nc.tensor.matmul(
                    psr[:], wt[:], ht[:, k, :], start=(k == 0), stop=(k == HKT - 1)
                )
            kraw = p1.tile([ROPE_D, SC], F32R)
            nc.vector.tensor_copy(kraw[:], psr[:])
            r64 = p1.tile([ROPE_D, ROPE_D], F32R)
            nc.sync.dma_start(r64[:], rot64[:].bitcast(F32R))
            rps = ps1.tile([ROPE_D, SC], F32, tag="psrot2")
            nc.tensor.matmul(rps[:], r64[:], kraw[:], start=True, stop=True)
            cct = p1.tile([ROPE_D, SC], F32)
            sct = p1.tile([ROPE_D, SC], F32)
            nc.sync.dma_start(cct[:], cos_c[:])
            nc.sync.dma_start(sct[:], sin_c[:])
            ktmp = p1.tile([ROPE_D, SC], F32)
            nc.vector.tensor_mul(ktmp[:], rps[:], sct[:])
            kfin = p1.tile([ROPE_D, SC], F32)
            nc.vector.tensor_mul(kfin[:], kraw[:], cct[:])
            nc.vector.tensor_add(kfin[:], kfin[:], ktmp[:])
            nc.sync.dma_start(ag_in[KR0 : KR0 + ROPE_D, :], kfin[:])

            # rstd rows
            sq1 = p1.tile([1, SC], F32)
            nc.scalar.activation(sq1[:], ssq[:], AF.Sqrt, scale=1.0 / Q_LORA, bias=epst[:])
            rq = p1.tile([1, SC], F32)
            nc.vector.reciprocal(rq[:], sq1[:])
            nc.sync.dma_start(ag_in[RSQ : RSQ + 1, :], rq[:])
            sq2 = p1.tile([1, SC], F32)
            nc.scalar.activation(sq2[:], sskv[:], AF.Sqrt, scale=1.0 / KV_LORA, bias=epst[:])
            rkv = p1.tile([1, SC], F32)
            nc.vector.reciprocal(rkv[:], sq2[:])
            nc.sync.dma_start(ag_in[RSKV : RSKV + 1, :], rkv[:])

        # ---------------- AllGather ----------------
        nc.gpsimd.collective_compute(
            "AllGather",
            mybir.AluOpType.bypass,
            replica_groups=[list(range(NCORE))],
            ins=[ag_in[:]],
            outs=[ag_out[:]],
        )
        if debug:
            nc.sync.dma_start(dbg["ag"][:], ag_out[:])

        # ---------------- phase 2 ----------------
        with (
            tc.tile_pool(name="glob", bufs=1) as G,
            tc.tile_pool(name="psS", bufs=2, space="PSUM") as psS,
        ):
            k_rot = G.tile([ROPE_D, S], F32R)
            rsq_bc = G.tile([128, S], F32)
            rskv_bc = G.tile([128, S], F32)
            with tc.tile_pool(name="rows", bufs=1) as RW:
                rsq_row = RW.tile([1, S], F32)
                rskv_row = RW.tile([1, S], F32)
                for c in range(NCORE):
                    base = c * AGR
                    nc.sync.dma_start(
                        k_rot[:, _cs(c)], ag_out[base + KR0 : base + KR0 + ROPE_D, :].bitcast(F32R)
                    )
                    nc.sync.dma_start(rsq_row[:, _cs(c)], ag_out[base + RSQ : base + RSQ + 1, :])
                    nc.sync.dma_start(rskv_row[:, _cs(c)], ag_out[base + RSKV : base + RSKV + 1, :])
                nc.gpsimd.partition_broadcast(rsq_bc[:], rsq_row[:])
                nc.gpsimd.partition_broadcast(rskv_bc[:], rskv_row[:])
            mask_sb = G.tile([128, 896], F32R)
            nc.sync.dma_start(mask_sb[:], mask_strip[:].bitcast(F32R))
            ones2 = G.tile([128, 1], F32R)
            nc.sync.dma_start(ones2[:], ones_in[:].bitcast(F32R))
            o_sb = G.tile([128, HPC, S], F32R)

            for p in range(2):  # head pairs
                with tc.tile_pool(name=f"ppk{p}", bufs=1) as PPK:
                  qn = PPK.tile([128, 2, S], F32R, name=f"qn{p}")
                  qr = PPK.tile([ROPE_D, 2, S], F32R, name=f"qr{p}")
                  kn = PPK.tile([128, 2, S], F32R, name=f"kn{p}")
                  vp = PPK.tile([128, 16, 256], F32R, name=f"vp{p}")
                  with (
                    tc.tile_pool(name=f"pp{p}", bufs=1) as PP,
                    tc.tile_pool(name=f"pps{p}", bufs=2, space="PSUM") as PPS,
                  ):
                    wqbn_p = PP.tile([128, QKT, 256], F32R, name=f"wqbn{p}")
                    wqbr_p = PP.tile([128, QKT, 128], F32R, name=f"wqbr{p}")
                    for k in range(QKT):
                        nc.sync.dma_start(
                            wqbn_p[:, k, :],
                            w_qb_nope_T[_kts(k), p * 256 : (p + 1) * 256].bitcast(F32R),
                        )
                        nc.sync.dma_start(
                            wqbr_p[:, k, :],
                            w_qb_rot_T[_kts(k), p * 128 : (p + 1) * 128].bitcast(F32R),
                        )
                    for c in range(NCORE):
                        ql = PP.tile([128, QKT, SC], F32R, tag="qlat", bufs=2)
                        for k in range(QKT):
                            nc.sync.dma_start(
                                ql[:, k, :],
                                ag_out[c * AGR + k * 128 : c * AGR + (k + 1) * 128, :].bitcast(F32R),
                            )
                        for h in range(2):
                            psn = PPS.tile([128, SC], F32, tag="psq", bufs=4)
                            for k in range(QKT):
                                nc.tensor.matmul(
                                    psn[:],
                                    wqbn_p[:, k, h * 128 : (h + 1) * 128],
                                    ql[:, k, :],
                                    start=(k == 0),
                                    stop=(k == QKT - 1),
                                )
                            nc.vector.tensor_mul(qn[:, h, _cs(c)], psn[:], rsq_bc[:, _cs(c)])
                            psrr = PPS.tile([ROPE_D, SC], F32, tag="psq", bufs=4)
                            for k in range(QKT):
                                nc.tensor.matmul(
                                    psrr[:],
                                    wqbr_p[:, k, h * 64 : (h + 1) * 64],
                                    ql[:, k, :],
                                    start=(k == 0),
                                    stop=(k == QKT - 1),
                                )
                            nc.vector.tensor_mul(
                                qr[:, h, _cs(c)], psrr[:], rsq_bc[0:ROPE_D, _cs(c)]
                            )

                    # rope on qr, in place
                    r64b = PP.tile([ROPE_D, ROPE_D], F32R, name=f"r64b{p}")
                    nc.sync.dma_start(r64b[:], rot64[:].bitcast(F32R))
                    for h in range(2):
                        for nb in range(4):
                            rps2 = PPS.tile([ROPE_D, 512], F32, tag="psq", bufs=4)
                            nc.tensor.matmul(
                                rps2[:], r64b[:], qr[:, h, _qs(nb)], start=True, stop=True
                            )
                            cct2 = PP.tile([ROPE_D, 512], F32, tag="cosl", bufs=3)
                            nc.sync.dma_start(cct2[:], cos_T[:, _qs(nb)])
                            sct2 = PP.tile([ROPE_D, 512], F32, tag="sinl", bufs=3)
                            nc.sync.dma_start(sct2[:], sin_T[:, _qs(nb)])
                            rtmp = PP.tile([ROPE_D, 512], F32, tag="rtmp", bufs=2)
                            nc.vector.tensor_mul(rtmp[:], rps2[:], sct2[:])
                            nc.vector.tensor_mul(qr[:, h, _qs(nb)], qr[:, h, _qs(nb)], cct2[:])
                            nc.vector.tensor_add(qr[:, h, _qs(nb)], qr[:, h, _qs(nb)], rtmp[:])

                    # kv projections
                    wkb_p = PP.tile([128, KVKT, 256], F32R, name=f"wkb{p}")
                    wvb_p = PP.tile([128, KVKT, 256], F32R, name=f"wvb{p}")
                    for k in range(KVKT):
                        nc.sync.dma_start(
                            wkb_p[:, k, :],
                            w_kb_T[_kts(k), p * 256 : (p + 1) * 256].bitcast(F32R),
                        )
                        nc.sync.dma_start(
                            wvb_p[:, k, :],
                            w_vb_T[_kts(k), p * 256 : (p + 1) * 256].bitcast(F32R),
                        )
                    for c in range(NCORE):
                        kvn = []
                        for k in range(KVKT):
                            kvt = PP.tile([128, SC], F32, tag="kvg", bufs=4)
                            nc.sync.dma_start(
                                kvt[:],
                                ag_out[c * AGR + KV0 + k * 128 : c * AGR + KV0 + (k + 1) * 128, :],
                            )
                            kvnk = PP.tile([128, SC], F32R, tag="kvn", bufs=5)
                            nc.vector.tensor_mul(kvnk[:], kvt[:], rskv_bc[:, _cs(c)])
                            kvn.append(kvnk)
                        for h in range(2):
                            pk = PPS.tile([128, SC], F32, tag="psq", bufs=4)
                            for k in range(KVKT):
                                nc.tensor.matmul(
                                    pk[:],
                                    wkb_p[:, k, h * 128 : (h + 1) * 128],
                                    kvn[k][:],
                                    start=(k == 0),
                                    stop=(k == KVKT - 1),
                                )
                            nc.vector.tensor_copy(kn[:, h, _cs(c)], pk[:])
                        for tt in range(2):
                            pv = PPS.tile([128, 256], F32, tag="psq", bufs=4)
                            for k in range(KVKT):
                                nc.tensor.matmul(
                                    pv[:],
                                    kvn[k][:, tt * 128 : (tt + 1) * 128],
                                    wvb_p[:, k, :],
                                    start=(k == 0),
                                    stop=(k == KVKT - 1),
                                )
                            nc.vector.tensor_copy(vp[:, c * 2 + tt, :], pv[:])

                  if debug and p == 0:
                    nc.sync.dma_start(dbg["qnope"][:], qn[:, 0, :].bitcast(F32))
                    nc.sync.dma_start(dbg["qrot"][:], qr[:, 0, :].bitcast(F32))
                    nc.sync.dma_start(dbg["knope"][:], kn[:, 0, :].bitcast(F32))
                    nc.sync.dma_start(
                        dbg["v"][:], vp[:].rearrange("p a b -> p (a b)").bitcast(F32)
                    )

                  # attention
                  if True:
                    with (
                        tc.tile_pool(name=f"pa{p}", bufs=1) as PA,
                        tc.tile_pool(name=f"pas{p}", bufs=2, space="PSUM") as PAS,
                    ):
                        for h in range(2):
                            hh = p * 2 + h
                            for qb in range(4):
                                kt_max = 4 * (qb + 1)
                                ops = PAS.tile([128, 512], F32, tag="ops", bufs=2)
                                sacc = PA.tile([128, 512], F32R, tag="sacc", bufs=2)
                                for kt in range(kt_max):
                                    sps = PAS.tile([128, 512], F32, tag="sps", bufs=3)
                                    nc.tensor.matmul(
                                        sps[:],
                                        kn[:, h, _kts(kt)],
                                        qn[:, h, _qs(qb)],
                                        start=True,
                                        stop=False,
                                    )
                                    nc.tensor.matmul(
                                        sps[:],
                                        k_rot[:, _kts(kt)],
                                        qr[:, h, _qs(qb)],
                                        start=False,
                                        stop=True,
                                    )
                                    pt = PA.tile([128, 512], F32R, tag="pt", bufs=3)
                                    nc.scalar.activation(pt[:], sps[:], AF.Exp, scale=SCALING)
                                    v_idx = kt - 4 * qb
                                    if v_idx >= 0:
                                        nc.vector.tensor_mul(
                                            pt[:],
                                            pt[:],
                                            mask_sb[:, 384 - v_idx * 128 : 896 - v_idx * 128],
                                        )
                                    nc.tensor.matmul(
                                        ops[:],
                                        vp[:, kt, h * 128 : (h + 1) * 128],
                                        pt[:],
                                        start=(kt == 0),
                                        stop=(kt == kt_max - 1),
                                    )
                                    if kt == 0:
                                        nc.vector.tensor_copy(sacc[:], pt[:])
                                    else:
                                        nc.vector.tensor_add(sacc[:], sacc[:], pt[:])
                                ssum = psS.tile([1, 512], F32, tag="ssum", bufs=2)
                                nc.tensor.matmul(ssum[:], ones2[:], sacc[:], start=True, stop=True)
                                rec = PA.tile([1, 512], F32, tag="rec", bufs=2)
                                nc.vector.reciprocal(rec[:], ssum[:])
                                rb = PA.tile([128, 512], F32, tag="rb", bufs=2)
                                nc.gpsimd.partition_broadcast(rb[:], rec[:])
                                nc.vector.tensor_mul(o_sb[:, hh, _qs(qb)], ops[:], rb[:])

            if debug:
                nc.sync.dma_start(dbg["o"][:], o_sb[:, 0, :].bitcast(F32))

            # o_proj
            with (
                tc.tile_pool(name="op", bufs=1) as OP,
                tc.tile_pool(name="pso", bufs=4, space="PSUM") as PSO,
            ):
                for m in range(HIDDEN // 128):
                    wo_t = OP.tile([128, HPC, 128], F32R, tag="wo", bufs=3)
                    for hh in range(HPC):
                        nc.sync.dma_start(
                            wo_t[:, hh, :],
                            w_o_T[hh * 128 : (hh + 1) * 128, m * 128 : (m + 1) * 128].bitcast(F32R),
                        )
                    for qb in range(4):
                        po = PSO.tile([128, 512], F32, tag="po")
                        for hh in range(HPC):
                            nc.tensor.matmul(
                                po[:],
                                wo_t[:, hh, :],
                                o_sb[:, hh, _qs(qb)],
                                start=(hh == 0),
                                stop=(hh == HPC - 1),
                            )
                        ost = OP.tile([128, 512], F32, tag="ost", bufs=4)
                        nc.vector.tensor_copy(ost[:], po[:])
                        nc.sync.dma_start(
                            o_partial[m * 128 : (m + 1) * 128, _qs(qb)], ost[:]
                        )

    nc.compile()
    return nc


def make_in_maps(
    hidden_states, cos, sin, w_q_a, q_a_ln_w, w_q_b, w_kv_a, kv_a_ln_w, w_kv_b, w_o
):
    f32 = np.float32
    hidden_T = np.ascontiguousarray(hidden_states[0].T, dtype=f32)  # [4096, 2048]
    w_q_a_T = np.ascontiguousarray(np.asarray(w_q_a).T, dtype=f32)
    w_kv_a_T = np.ascontiguousarray(np.asarray(w_kv_a).T, dtype=f32)
    cos_T = np.ascontiguousarray(np.asarray(cos)[0].T, dtype=f32)  # [64, 2048]
    sin_T = np.ascontiguousarray(np.asarray(sin)[0].T, dtype=f32)

    w_qb_eff = np.asarray(w_q_b, dtype=f32) * np.asarray(q_a_ln_w, dtype=f32)[None, :]
    w_kvb_eff = np.asarray(w_kv_b, dtype=f32) * np.asarray(kv_a_ln_w, dtype=f32)[None, :]
    w_o = np.asarray(w_o, dtype=f32)

    R = np.zeros((ROPE_D, ROPE_D), dtype=f32)
    half = ROPE_D // 2
    R[np.arange(half), np.arange(half) + half] = -1.0
    R[np.arange(half) + half, np.arange(half)] = 1.0
    rot64 = np.ascontiguousarray(R.T)

    ones_in = np.ones((128, 1), dtype=f32)
    xs = np.arange(896)[None, :] - 384
    ps = np.arange(128)[:, None]
    mask_strip = (xs >= ps).astype(f32)

    in_maps = []
    for c in range(NCORE):
        heads = range(HPC * c, HPC * (c + 1))
        qb_nope = np.concatenate(
            [w_qb_eff[h * QH : h * QH + NOPE_D, :] for h in heads], axis=0
        )  # [512, 1536]
        qb_rot = np.concatenate(
            [w_qb_eff[h * QH + NOPE_D : (h + 1) * QH, :] for h in heads], axis=0
        )  # [256, 1536]
        kb = np.concatenate(
            [w_kvb_eff[h * 256 : h * 256 + NOPE_D, :] for h in heads], axis=0
        )  # [512, 512]
        vb = np.concatenate(
            [w_kvb_eff[h * 256 + NOPE_D : (h + 1) * 256, :] for h in heads], axis=0
        )  # [512, 512]
        wo_c = w_o[:, c * HPC * V_D : (c + 1) * HPC * V_D]  # [4096, 512]
        in_maps.append(
            {
                "hidden_T": np.ascontiguousarray(hidden_T[:, c * SC : (c + 1) * SC]),
                "w_q_a_T": w_q_a_T,
                "w_kv_a_T": w_kv_a_T,
                "cos_c": np.ascontiguousarray(cos_T[:, c * SC : (c + 1) * SC]),
                "sin_c": np.ascontiguousarray(sin_T[:, c * SC : (c + 1) * SC]),
                "cos_T": cos_T,
                "sin_T": sin_T,
                "w_qb_nope_T": np.ascontiguousarray(qb_nope.T),
                "w_qb_rot_T": np.ascontiguousarray(qb_rot.T),
                "w_kb_T": np.ascontiguousarray(kb.T),
                "w_vb_T": np.ascontiguousarray(vb.T),
                "w_o_T": np.ascontiguousarray(wo_c.T),
                "rot64": rot64,
                "ones_in": ones_in,
                "eps_in": np.full((1, 1), EPS, dtype=f32),
                "mask_strip": mask_strip,
            }
        )
    return in_maps


_NC_CACHE = {}


def _get_nc(debug=False):
    if debug not in _NC_CACHE:
        _NC_CACHE[debug] = build(debug=debug)
    return _NC_CACHE[debug]


def run(inputs, debug=False):
    nc = _get_nc(debug=debug)
    in_maps = make_in_maps(**inputs)
    res = run_bass_kernel_spmd(nc, in_maps, core_ids=list(range(NCORE)))
    return res


def kernel(**inputs) -> np.ndarray:
    res = run(inputs)
    acc = np.zeros((HIDDEN, S), dtype=np.float64)
    for c in range(NCORE):
        acc += res.results[c]["o_partial"]
    return np.ascontiguousarray(acc.T, dtype=np.float32).reshape(1, S, HIDDEN)


if __name__ == "__main__":
    import reference

    inputs = {k: np.asarray(v) for k, v in reference.setup_inputs().items()}
    out = kernel(**inputs)
    exp = np.asarray(reference.reference(**reference.setup_inputs()))
    rel = np.linalg.norm(out - exp) / np.linalg.norm(exp)
    print("Relative error:", rel)


# revision 6
# speedup vs baseline: 1.8696x; 1.8696x over previous
"""DeepSeek-V3 MLA attention (B=1, S=2048) on 8 TRN2 NeuronCores.

Sharding: tensor-parallel over heads (4 heads/core) for the up-projections,
attention and o_proj; the low-rank down-projections are token-sharded
(256 tokens/core) and exchanged with a single AllGather of the latents.
Each core returns a partial o_proj output (its 4 heads); the host sums them.

All on-chip matmuls run as float32r (full PE rate). Activations are kept
feature-major [feat, token]; softmax runs on transposed scores
S^T[tok_k, tok_q] so the reduction over keys is a ones-vector matmul.
"""

from contextlib import ExitStack

import numpy as np

import concourse.bass as bass
import concourse.tile as tile
from concourse import bacc, mybir
from concourse.bass_utils import run_bass_kernel_spmd

F32 = mybir.dt.float32
F32R = mybir.dt.float32r
AF = mybir.ActivationFunctionType

HIDDEN = 4096
N_HEADS = 32
Q_LORA = 1536
KV_LORA = 512
ROPE_D = 64
NOPE_D = 128
V_D = 128
QH = NOPE_D + ROPE_D  # 192
EPS = 1e-6
SCALING = QH ** -0.5
S = 2048
NCORE = 8
SC = S // NCORE  # 256 tokens per core chunk
HPC = N_HEADS // NCORE  # 4 heads per core

QKT = Q_LORA // 128  # 12
KVKT = KV_LORA // 128  # 4
HKT = HIDDEN // 128  # 32

# AllGather contribution rows: q_lat(1536) + kv_lat(512) + k_rot(64) + 2 rstd rows
KV0 = Q_LORA
KR0 = Q_LORA + KV_LORA
RSQ = KR0 + ROPE_D
RSKV = RSQ + 1
AGR = RSKV + 1  # 2114


def _cs(c):
    return slice(c * SC, (c + 1) * SC)


def _qs(qb):
    return slice(qb * 512, (qb + 1) * 512)


def _kts(kt):
    return slice(kt * 128, (kt + 1) * 128)


def build(debug=False):
    nc = bacc.Bacc(None, target_bir_lowering=False, num_devices=NCORE)

    def din(name, shape):
        return nc.dram_tensor(name, shape, F32, kind="ExternalInput")

    hidden_T = din("hidden_T", [HIDDEN, SC])
    w_q_a_T = din("w_q_a_T", [HIDDEN, Q_LORA])
    w_kv_a_T = din("w_kv_a_T", [HIDDEN, KV_LORA + ROPE_D])
    cos_c = din("cos_c", [ROPE_D, SC])
    sin_c = din("sin_c", [ROPE_D, SC])
    cos_T = din("cos_T", [ROPE_D, S])
    sin_T = din("sin_T", [ROPE_D, S])
    w_qb_nope_T = din("w_qb_nope_T", [Q_LORA, HPC * NOPE_D])
    w_qb_rot_T = din("w_qb_rot_T", [Q_LORA, HPC * ROPE_D])
    w_kb_T = din("w_kb_T", [KV_LORA, HPC * NOPE_D])
    w_vb_T = din("w_vb_T", [KV_LORA, HPC * V_D])
    w_o_T = din("w_o_T", [HPC * V_D, HIDDEN])
    rot64 = din("rot64", [ROPE_D, ROPE_D])
    ones_in = din("ones_in", [128, 1])
    eps_in = din("eps_in", [1, 1])
    mask_strip = din("mask_strip", [128, 896])

    o_partial = nc.dram_tensor("o_partial", [HIDDEN, S], F32, kind="ExternalOutput")
    dbg = {}
    if debug:
        dbg["ag"] = nc.dram_tensor("dbg_ag", [NCORE * AGR, SC], F32, kind="ExternalOutput")
        dbg["qnope"] = nc.dram_tensor("dbg_qnope", [NOPE_D, S], F32, kind="ExternalOutput")
        dbg["qrot"] = nc.dram_tensor("dbg_qrot", [ROPE_D, S], F32, kind="ExternalOutput")
        dbg["knope"] = nc.dram_tensor("dbg_knope", [NOPE_D, S], F32, kind="ExternalOutput")
        dbg["v"] = nc.dram_tensor("dbg_v", [128, 16 * 256], F32, kind="ExternalOutput")
        dbg["o"] = nc.dram_tensor("dbg_o", [V_D, S], F32, kind="ExternalOutput")

    ag_in = nc.dram_tensor("ag_in", [AGR, SC], F32, kind="Internal")
    ag_out = nc.dram_tensor(
        "ag_out", [NCORE * AGR, SC], F32, kind="Internal", addr_space="Shared"
    )

    with tile.TileContext(nc) as tc:
        # ---------------- phase 1: latents for own token chunk ----------------
        with (
            tc.tile_pool(name="p1", bufs=1) as p1,
            tc.tile_pool(name="p1s", bufs=8) as p1s,
            tc.tile_pool(name="ps1", bufs=2, space="PSUM") as ps1,
            tc.tile_pool(name="ps1a", bufs=1, space="PSUM") as ps1a,
        ):
            ht = p1.tile([128, HKT, SC], F32R)
            nc.sync.dma_start(
                ht[:], hidden_T[:].rearrange("(t p) n -> p t n", p=128).bitcast(F32R)
            )
            ones1 = p1.tile([128, 1], F32R)
            nc.sync.dma_start(ones1[:], ones_in[:].bitcast(F32R))
            epst = p1.tile([1, 1], F32)
            nc.sync.dma_start(epst[:], eps_in[:])

            ssq = ps1a.tile([1, SC], F32)
            sskv = ps1a.tile([1, SC], F32)

            wqa_r = w_q_a_T[:].rearrange("(k p) m -> p k m", p=128).bitcast(F32R)
            wkva_r = w_kv_a_T[:].rearrange("(k p) m -> p k m", p=128).bitcast(F32R)
            # q_lat m-tiles
            for m in range(QKT):
                ps = ps1.tile([128, SC], F32, tag="pslat")
                wt = p1s.tile([128, HKT, 128], F32R, tag="wqa", bufs=2)
                nc.sync.dma_start(wt[:], wqa_r[:, :, m * 128 : (m + 1) * 128])
                for k in range(HKT):
                    nc.tensor.matmul(
                        ps[:], wt[:, k, :], ht[:, k, :], start=(k == 0), stop=(k == HKT - 1)
                    )
                raw = p1s.tile([128, SC], F32, tag="lraw")
                nc.vector.tensor_copy(raw[:], ps[:])
                nc.sync.dma_start(ag_in[m * 128 : (m + 1) * 128, :], raw[:])
                sq = p1s.tile([128, SC], F32R, tag="lsq")
                nc.scalar.activation(sq[:], ps[:], AF.Square)
                nc.tensor.matmul(
                    ssq[:], ones1[:], sq[:], start=(m == 0), stop=(m == QKT - 1)
                )

            # kv_lat m-tiles
            for m in range(KVKT):
                ps = ps1.tile([128, SC], F32, tag="pslat")
                wt = p1s.tile([128, HKT, 128], F32R, tag="wqa", bufs=2)
                nc.sync.dma_start(wt[:], wkva_r[:, :, m * 128 : (m + 1) * 128])
                for k in range(HKT):
                    nc.tensor.matmul(
                        ps[:], wt[:, k, :], ht[:, k, :], start=(k == 0), stop=(k == HKT - 1)
                    )
                raw = p1s.tile([128, SC], F32, tag="lraw")
                nc.vector.tensor_copy(raw[:], ps[:])
                nc.sync.dma_start(ag_in[KV0 + m * 128 : KV0 + (m + 1) * 128, :], raw[:])
                sq = p1s.tile([128, SC], F32R, tag="lsq")
                nc.scalar.activation(sq[:], ps[:], AF.Square)
                nc.tensor.matmul(
                    sskv[:], ones1[:], sq[:], start=(m == 0), stop=(m == KVKT - 1)
                )

            # k_rot (rows 512:576 of ckv), roped with this chunk's cos/sin
            psr = ps1.tile([ROPE_D, SC], F32, tag="psrot")
            wtr = p1s.tile([128, HKT, ROPE_D], F32R, tag="wkvr", bufs=1)
            nc.sync.dma_start(wtr[:], wkva_r[:, :, KV_LORA : KV_LORA + ROPE_D])
            for k in range(HKT):
                nc.tensor.matmul(
                    psr[:], wtr[:, k, :], ht[:, k, :], start=(k == 0), stop=(k == HKT - 1)
                )
            kraw = p1.tile([ROPE_D, SC], F32R)
            nc.vector.tensor_copy(kraw[:], psr[:])
            r64 = p1.tile([ROPE_D, ROPE_D], F32R)
            nc.sync.dma_start(r64[:], rot64[:].bitcast(F32R))
            rps = ps1.tile([ROPE_D, SC], F32, tag="psrot2")
            nc.tensor.matmul(rps[:], r64[:], kraw[:], start=True, stop=True)
            cct = p1.tile([ROPE_D, SC], F32)
            sct = p1.tile([ROPE_D, SC], F32)
            nc.sync.dma_start(cct[:], cos_c[:])
            nc.sync.dma_start(sct[:], sin_c[:])
            ktmp = p1.tile([ROPE_D, SC], F32)
            nc.vector.tensor_mul(ktmp[:], rps[:], sct[:])
            kfin = p1.tile([ROPE_D, SC], F32)
            nc.vector.tensor_mul(kfin[:], kraw[:], cct[:])
            nc.vector.tensor_add(kfin[:], kfin[:], ktmp[:])
            nc.sync.dma_start(ag_in[KR0 : KR0 + ROPE_D, :], kfin[:])

            # rstd rows
            sq1 = p1.tile([1, SC], F32)
            nc.scalar.activation(sq1[:], ssq[:], AF.Sqrt, scale=1.0 / Q_LORA, bias=epst[:])
            rq = p1.tile([1, SC], F32)
            nc.vector.reciprocal(rq[:], sq1[:])
            nc.sync.dma_start(ag_in[RSQ : RSQ + 1, :], rq[:])
            sq2 = p1.tile([1, SC], F32)
            nc.scalar.activation(sq2[:], sskv[:], AF.Sqrt, scale=1.0 / KV_LORA, bias=epst[:])
            rkv = p1.tile([1, SC], F32)
            nc.vector.reciprocal(rkv[:], sq2[:])
            nc.sync.dma_start(ag_in[RSKV : RSKV + 1, :], rkv[:])

        # ---------------- AllGather ----------------
        nc.gpsimd.collective_compute(
            "AllGather",
            mybir.AluOpType.bypass,
            replica_groups=[list(range(NCORE))],
            ins=[ag_in[:]],
            outs=[ag_out[:]],
        )
        if debug:
            nc.sync.dma_start(dbg["ag"][:], ag_out[:])

        # ---------------- phase 2 ----------------
        with (
            tc.tile_pool(name="glob", bufs=1) as G,
            tc.tile_pool(name="psS", bufs=2, space="PSUM") as psS,
        ):
            k_rot = G.tile([ROPE_D, S], F32R)
            rsq_bc = G.tile([128, S], F32)
            rskv_bc = G.tile([128, S], F32)
            with tc.tile_pool(name="rows", bufs=1) as RW:
                rsq_row = RW.tile([1, S], F32)
                rskv_row = RW.tile([1, S], F32)
                for c in range(NCORE):
                    base = c * AGR
                    nc.sync.dma_start(
                        k_rot[:, _cs(c)], ag_out[base + KR0 : base + KR0 + ROPE_D, :].bitcast(F32R)
                    )
                    nc.sync.dma_start(rsq_row[:, _cs(c)], ag_out[base + RSQ : base + RSQ + 1, :])
                    nc.sync.dma_start(rskv_row[:, _cs(c)], ag_out[base + RSKV : base + RSKV + 1, :])
                nc.gpsimd.partition_broadcast(rsq_bc[:], rsq_row[:])
                nc.gpsimd.partition_broadcast(rskv_bc[:], rskv_row[:])
            mask_sb = G.tile([128, 896], F32R)
            nc.sync.dma_start(mask_sb[:], mask_strip[:].bitcast(F32R))
            ones2 = G.tile([128, 1], F32R)
            nc.sync.dma_start(ones2[:], ones_in[:].bitcast(F32R))
            o_sb = G.tile([128, HPC, S], F32R)

            for p in range(2):  # head pairs
                with tc.tile_pool(name=f"ppk{p}", bufs=1) as PPK:
                  qn = PPK.tile([128, 2, S], F32R, name=f"qn{p}")
                  qr = PPK.tile([ROPE_D, 2, S], F32R, name=f"qr{p}")
                  kn = PPK.tile([128, 2, S], F32R, name=f"kn{p}")
                  vp = PPK.tile([128, 16, 256], F32R, name=f"vp{p}")
                  with (
                    tc.tile_pool(name=f"pp{p}", bufs=1) as PP,
                    tc.tile_pool(name=f"pps{p}", bufs=2, space="PSUM") as PPS,
                  ):
                    wqbn_p = PP.tile([128, QKT, 256], F32R, name=f"wqbn{p}")
                    wqbr_p = PP.tile([128, QKT, 128], F32R, name=f"wqbr{p}")
                    nc.sync.dma_start(
                        wqbn_p[:],
                        w_qb_nope_T[:].rearrange("(k p) m -> p k m", p=128)[
                            :, :, p * 256 : (p + 1) * 256
                        ].bitcast(F32R),
                    )
                    nc.sync.dma_start(
                        wqbr_p[:],
                        w_qb_rot_T[:].rearrange("(k p) m -> p k m", p=128)[
                            :, :, p * 128 : (p + 1) * 128
                        ].bitcast(F32R),
                    )
                    for c in range(NCORE):
                        ql = PP.tile([128, QKT, SC], F32R, tag="qlat", bufs=2)
                        nc.sync.dma_start(
                            ql[:],
                            ag_out[c * AGR : c * AGR + Q_LORA, :]
                            .rearrange("(k p) n -> p k n", p=128)
                            .bitcast(F32R),
                        )
                        for h in range(2):
                            psn = PPS.tile([128, SC], F32, tag="psq", bufs=4)
                            for k in range(QKT):
                                nc.tensor.matmul(
                                    psn[:],
                                    wqbn_p[:, k, h * 128 : (h + 1) * 128],
                                    ql[:, k, :],
                                    start=(k == 0),
                                    stop=(k == QKT - 1),
                                )
                            nc.vector.tensor_mul(qn[:, h, _cs(c)], psn[:], rsq_bc[:, _cs(c)])
                            psrr = PPS.tile([ROPE_D, SC], F32, tag="psq", bufs=4)
                            for k in range(QKT):
                                nc.tensor.matmul(
                                    psrr[:],
                                    wqbr_p[:, k, h * 64 : (h + 1) * 64],
                                    ql[:, k, :],
                                    start=(k == 0),
                                    stop=(k == QKT - 1),
                                )
                            nc.vector.tensor_mul(
                                qr[:, h, _cs(c)], psrr[:], rsq_bc[0:ROPE_D, _cs(c)]
                            )

                    # rope on qr, in place
                    r64b = PP.tile([ROPE_D, ROPE_D], F32R, name=f"r64b{p}")
                    nc.sync.dma_start(r64b[:], rot64[:].bitcast(F32R))
                    for h in range(2):
                        for nb in range(4):
                            rps2 = PPS.tile([ROPE_D, 512], F32, tag="psq", bufs=4)
                            nc.tensor.matmul(
                                rps2[:], r64b[:], qr[:, h, _qs(nb)], start=True, stop=True
                            )
                            cct2 = PP.tile([ROPE_D, 512], F32, tag="cosl", bufs=3)
                            nc.sync.dma_start(cct2[:], cos_T[:, _qs(nb)])
                            sct2 = PP.tile([ROPE_D, 512], F32, tag="sinl", bufs=3)
                            nc.sync.dma_start(sct2[:], sin_T[:, _qs(nb)])
                            rtmp = PP.tile([ROPE_D, 512], F32, tag="rtmp", bufs=2)
                            nc.vector.tensor_mul(rtmp[:], rps2[:], sct2[:])
                            nc.vector.tensor_mul(qr[:, h, _qs(nb)], qr[:, h, _qs(nb)], cct2[:])
                            nc.vector.tensor_add(qr[:, h, _qs(nb)], qr[:, h, _qs(nb)], rtmp[:])

                    # kv projections
                    wkb_p = PP.tile([128, KVKT, 256], F32R, name=f"wkb{p}")
                    wvb_p = PP.tile([128, KVKT, 256], F32R, name=f"wvb{p}")
                    nc.sync.dma_start(
                        wkb_p[:],
                        w_kb_T[:].rearrange("(k p) m -> p k m", p=128)[
                            :, :, p * 256 : (p + 1) * 256
                        ].bitcast(F32R),
                    )
                    nc.sync.dma_start(
                        wvb_p[:],
                        w_vb_T[:].rearrange("(k p) m -> p k m", p=128)[
                            :, :, p * 256 : (p + 1) * 256
                        ].bitcast(F32R),
                    )
                    for c in range(NCORE):
                        kvt = PP.tile([128, KVKT, SC], F32, tag="kvg", bufs=2)
                        nc.sync.dma_start(
                            kvt[:],
                            ag_out[c * AGR + KV0 : c * AGR + KV0 + KV_LORA, :].rearrange(
                                "(k p) n -> p k n", p=128
                            ),
                        )
                        kvn = []
                        for k in range(KVKT):
                            kvnk = PP.tile([128, SC], F32R, tag="kvn", bufs=5)
                            nc.vector.tensor_mul(kvnk[:], kvt[:, k, :], rskv_bc[:, _cs(c)])
                            kvn.append(kvnk)
                        for h in range(2):
                            pk = PPS.tile([128, SC], F32, tag="psq", bufs=4)
                            for k in range(KVKT):
                                nc.tensor.matmul(
                                    pk[:],
                                    wkb_p[:, k, h * 128 : (h + 1) * 128],
                                    kvn[k][:],
                                    start=(k == 0),
                                    stop=(k == KVKT - 1),
                                )
                            nc.vector.tensor_copy(kn[:, h, _cs(c)], pk[:])
                        for tt in range(2):
                            pv = PPS.tile([128, 256], F32, tag="psq", bufs=4)
                            for k in range(KVKT):
                                nc.tensor.matmul(
                                    pv[:],
                                    kvn[k][:, tt * 128 : (tt + 1) * 128],
                                    wvb_p[:, k, :],
                                    start=(k == 0),
                                    stop=(k == KVKT - 1),
                                )
                            nc.vector.tensor_copy(vp[:, c * 2 + tt, :], pv[:])

                  if debug and p == 0:
                    nc.sync.dma_start(dbg["qnope"][:], qn[:, 0, :].bitcast(F32))
                    nc.sync.dma_start(dbg["qrot"][:], qr[:, 0, :].bitcast(F32))
                    nc.sync.dma_start(dbg["knope"][:], kn[:, 0, :].bitcast(F32))
                    nc.sync.dma_start(
                        dbg["v"][:], vp[:].rearrange("p a b -> p (a b)").bitcast(F32)
                    )

                  # attention
                  if True:
                    with (
                        tc.tile_pool(name=f"pa{p}", bufs=1) as PA,
                        tc.tile_pool(name=f"pas{p}", bufs=2, space="PSUM") as PAS,
                    ):
                        for h in range(2):
                            hh = p * 2 + h
                            for qb in range(4):
                                kt_max = 4 * (qb + 1)
                                ops = PAS.tile([128, 512], F32, tag="ops", bufs=2)
                                ssum = psS.tile([1, 512], F32, tag="ssum", bufs=2)
                                for kt in range(kt_max):
                                    sps = PAS.tile([128, 512], F32, tag="sps", bufs=3)
                                    nc.tensor.matmul(
                                        sps[:],
                                        kn[:, h, _kts(kt)],
                                        qn[:, h, _qs(qb)],
                                        start=True,
                                        stop=False,
                                    )
                                    nc.tensor.matmul(
                                        sps[:],
                                        k_rot[:, _kts(kt)],
                                        qr[:, h, _qs(qb)],
                                        start=False,
                                        stop=True,
                                    )
                                    pt = PA.tile([128, 512], F32R, tag="pt", bufs=3)
                                    nc.scalar.activation(pt[:], sps[:], AF.Exp, scale=SCALING)
                                    v_idx = kt - 4 * qb
                                    if v_idx >= 0:
                                        nc.vector.tensor_mul(
                                            pt[:],
                                            pt[:],
                                            mask_sb[:, 384 - v_idx * 128 : 896 - v_idx * 128],
                                        )
                                    nc.tensor.matmul(
                                        ops[:],
                                        vp[:, kt, h * 128 : (h + 1) * 128],
                                        pt[:],
                                        start=(kt == 0),
                                        stop=(kt == kt_max - 1),
                                    )
                                    nc.tensor.matmul(
                                        ssum[:],
                                        ones2[:],
                                        pt[:],
                                        start=(kt == 0),
                                        stop=(kt == kt_max - 1),
                                    )
                                rec = PA.tile([1, 512], F32, tag="rec", bufs=2)
                                nc.vector.reciprocal(rec[:], ssum[:])
                                rb = PA.tile([128, 512], F32, tag="rb", bufs=2)
                                nc.gpsimd.partition_broadcast(rb[:], rec[:])
                                nc.vector.tensor_mul(o_sb[:, hh, _qs(qb)], ops[:], rb[:])

            if debug:
                nc.sync.dma_start(dbg["o"][:], o_sb[:, 0, :].bitcast(F32))

            # o_proj
            with (
                tc.tile_pool(name="op", bufs=1) as OP,
                tc.tile_pool(name="pso", bufs=4, space="PSUM") as PSO,
            ):
                wo_r = w_o_T[:].rearrange("(h p) m -> p h m", p=128).bitcast(F32R)
                for m in range(HIDDEN // 128):
                    wo_t = OP.tile([128, HPC, 128], F32R, tag="wo", bufs=3)
                    nc.sync.dma_start(wo_t[:], wo_r[:, :, m * 128 : (m + 1) * 128])
                    ost = OP.tile([128, 4, 512], F32, tag="ost", bufs=2)
                    for qb in range(4):
                        po = PSO.tile([128, 512], F32, tag="po")
                        for hh in range(HPC):
                            nc.tensor.matmul(
                                po[:],
                                wo_t[:, hh, :],
                                o_sb[:, hh, _qs(qb)],
                                start=(hh == 0),
                                stop=(hh == HPC - 1),
                            )
                        if qb % 2 == 0:
                            nc.vector.tensor_copy(ost[:, qb, :], po[:])
                        else:
                            nc.scalar.copy(ost[:, qb, :], po[:])
                    nc.sync.dma_start(
                        o_partial[m * 128 : (m + 1) * 128, :],
                        ost[:].rearrange("p a b -> p (a b)"),
                    )

    nc.compile()
    return nc


def make_in_maps(
    hidden_states, cos, sin, w_q_a, q_a_ln_w, w_q_b, w_kv_a, kv_a_ln_w, w_kv_b, w_o
):
    f32 = np.float32
    hidden_T = np.ascontiguousarray(hidden_states[0].T, dtype=f32)  # [4096, 2048]
    w_q_a_T = np.ascontiguousarray(np.asarray(w_q_a).T, dtype=f32)
    w_kv_a_T = np.ascontiguousarray(np.asarray(w_kv_a).T, dtype=f32)
    cos_T = np.ascontiguousarray(np.asarray(cos)[0].T, dtype=f32)  # [64, 2048]
    sin_T = np.ascontiguousarray(np.asarray(sin)[0].T, dtype=f32)

    w_qb_eff = np.asarray(w_q_b, dtype=f32) * np.asarray(q_a_ln_w, dtype=f32)[None, :]
    w_kvb_eff = np.asarray(w_kv_b, dtype=f32) * np.asarray(kv_a_ln_w, dtype=f32)[None, :]
    w_o = np.asarray(w_o, dtype=f32)

    R = np.zeros((ROPE_D, ROPE_D), dtype=f32)
    half = ROPE_D // 2
    R[np.arange(half), np.arange(half) + half] = -1.0
    R[np.arange(half) + half, np.arange(half)] = 1.0
    rot64 = np.ascontiguousarray(R.T)

    ones_in = np.ones((128, 1), dtype=f32)
    xs = np.arange(896)[None, :] - 384
    ps = np.arange(128)[:, None]
    mask_strip = (xs >= ps).astype(f32)

    in_maps = []
    for c in range(NCORE):
        heads = range(HPC * c, HPC * (c + 1))
        qb_nope = np.concatenate(
            [w_qb_eff[h * QH : h * QH + NOPE_D, :] for h in heads], axis=0
        )  # [512, 1536]
        qb_rot = np.concatenate(
            [w_qb_eff[h * QH + NOPE_D : (h + 1) * QH, :] for h in heads], axis=0
        )  # [256, 1536]
        kb = np.concatenate(
            [w_kvb_eff[h * 256 : h * 256 + NOPE_D, :] for h in heads], axis=0
        )  # [512, 512]
        vb = np.concatenate(
            [w_kvb_eff[h * 256 + NOPE_D : (h + 1) * 256, :] for h in heads], axis=0
        )  # [512, 512]
        wo_c = w_o[:, c * HPC * V_D : (c + 1) * HPC * V_D]  # [4096, 512]
        in_maps.append(
            {
                "hidden_T": np.ascontiguousarray(hidden_T[:, c * SC : (c + 1) * SC]),
                "w_q_a_T": w_q_a_T,
                "w_kv_a_T": w_kv_a_T,
                "cos_c": np.ascontiguousarray(cos_T[:, c * SC : (c + 1) * SC]),
                "sin_c": np.ascontiguousarray(sin_T[:, c * SC : (c + 1) * SC]),
                "cos_T": cos_T,
                "sin_T": sin_T,
                "w_qb_nope_T": np.ascontiguousarray(qb_nope.T),
                "w_qb_rot_T": np.ascontiguousarray(qb_rot.T),
                "w_kb_T": np.ascontiguousarray(kb.T),
                "w_vb_T": np.ascontiguousarray(vb.T),
                "w_o_T": np.ascontiguousarray(wo_c.T),
                "rot64": rot64,
                "ones_in": ones_in,
                "eps_in": np.full((1, 1), EPS, dtype=f32),
                "mask_strip": mask_strip,
            }
        )
    return in_maps


_NC_CACHE = {}


def _get_nc(debug=False):
    if debug not in _NC_CACHE:
        _NC_CACHE[debug] = build(debug=debug)
    return _NC_CACHE[debug]


def run(inputs, debug=False):
    nc = _get_nc(debug=debug)
    in_maps = make_in_maps(**inputs)
    res = run_bass_kernel_spmd(nc, in_maps, core_ids=list(range(NCORE)))
    return res


def kernel(**inputs) -> np.ndarray:
    res = run(inputs)
    acc = np.zeros((HIDDEN, S), dtype=np.float64)
    for c in range(NCORE):
        acc += res.results[c]["o_partial"]
    return np.ascontiguousarray(acc.T, dtype=np.float32).reshape(1, S, HIDDEN)


if __name__ == "__main__":
    import reference

    inputs = {k: np.asarray(v) for k, v in reference.setup_inputs().items()}
    out = kernel(**inputs)
    exp = np.asarray(reference.reference(**reference.setup_inputs()))
    rel = np.linalg.norm(out - exp) / np.linalg.norm(exp)
    print("Relative error:", rel)
